# revision 18
# baseline (speedup 1.0000x reference)
"""Causal self-attention Trainium2 kernel (B=4, T=2048, C=1024, H=16, Dh=64).

Sharding: 8 cores = (batch b) x (head-group g of 8 heads).
Per-core: transposed flash attention in float32r (see DESIGN.md).
Returns (out [B,T,C] f32, attn_weights [B,H,T,T] f32) like the reference.
"""

import sys

sys.path.insert(0, "/opt/trn_rl_repo")

import numpy as np

import concourse.bass as bass  # noqa: F401
import concourse.mybir as mybir
from concourse import bacc
from concourse.bass_utils import run_bass_kernel_spmd
from concourse.tile import TileContext

F32 = mybir.dt.float32
F32R = mybir.dt.float32r
F16 = mybir.dt.float16
AF = mybir.ActivationFunctionType
ALU = mybir.AluOpType

B, T, C = 4, 2048, 1024
H, DH = 16, 64
HG = 8          # heads per core
NPAIR = 4       # head pairs per core
P = 128
TB = 512        # q-block width
NJ = T // TB    # 4 q-blocks
NKC = T // P    # 16 k-chunks
CCH = C // P    # 8 contraction chunks

TRACE = False
LAST_EXEC_NS = None
_NC_CACHE = {}


def _round_f32r(x):
    """Round fp32 to 13-bit mantissa (round-to-nearest-even), matching DVE."""
    b = np.ascontiguousarray(x, dtype=np.float32).view(np.uint32)
    lsb = (b >> 10) & np.uint32(1)
    b = b + np.uint32(0x1FF) + lsb
    b &= np.uint32(0xFFFFFC00)
    return b.view(np.float32)


def _ts(i, s):
    return slice(i * s, (i + 1) * s)


def _build_module():
    nc = bacc.Bacc("TRN2")

    xt_d = nc.dram_tensor("xt", [C, T], F32R, kind="ExternalInput")
    wqk_d = nc.dram_tensor("wqk", [C, 1024], F32R, kind="ExternalInput")
    wv_d = nc.dram_tensor("wv", [C, 512], F32R, kind="ExternalInput")
    wo_d = nc.dram_tensor("wo", [512, C], F32R, kind="ExternalInput")
    bqk_d = nc.dram_tensor("bqk", [P, 8], F32, kind="ExternalInput")
    bv_d = nc.dram_tensor("bv", [1, 512], F32R, kind="ExternalInput")
    tri_d = nc.dram_tensor("tri", [P, P], F16, kind="ExternalInput")
    ones_d = nc.dram_tensor("ones", [P, P], F32R, kind="ExternalInput")

    attn_d = nc.dram_tensor("attn_t", [HG, NKC // 2, NJ, P, 2, TB], F16, kind="ExternalOutput")
    yt_d = nc.dram_tensor("yt", [C, T], F32, kind="ExternalOutput")
    sums_d = nc.dram_tensor("sums", [HG, NJ, TB], F32, kind="ExternalOutput")

    xt_v = xt_d.rearrange("(c p) t -> c p t", p=P)          # [8,128,2048]
    wqk_v = wqk_d.rearrange("(c p) (f j) -> c p f j", p=P, j=P)  # [8,128,8,128]
    wv_v = wv_d.rearrange("(c p) v -> c p v", p=P)          # [8,128,512]
    wo_v = wo_d.rearrange("(m p) o -> m p o", p=P)          # [4,128,1024]
    yt_v = yt_d.rearrange("(o p) t -> o p t", p=P)          # [8,128,2048]

    with TileContext(nc) as tc, nc.allow_low_precision(reason="fp32r attention"):
        import contextlib

        with contextlib.ExitStack() as ctx:
            cpool = ctx.enter_context(tc.tile_pool(name="consts", bufs=1))
            qkpool = ctx.enter_context(tc.tile_pool(name="qk", bufs=1))
            vpool = ctx.enter_context(tc.tile_pool(name="vp", bufs=1))

            tri_sb = cpool.tile([P, P], F16, tag="tri")
            ones_sb = cpool.tile([P, P], F32R, tag="ones")
            bqk_sb = cpool.tile([P, 8], F32, tag="bqk")
            bv_sb = cpool.tile([1, 512], F32R, tag="bv")
            nc.sync.dma_start(tri_sb[:], tri_d[:])
            nc.sync.dma_start(ones_sb[:], ones_d[:])
            nc.sync.dma_start(bqk_sb[:], bqk_d[:])
            nc.sync.dma_start(bv_sb[:], bv_d[:])

            # Persistent activation tiles
            qt = [qkpool.tile([P, T], F32R, tag=f"qt{m}", name=f"qt{m}") for m in range(NPAIR)]
            kt = [qkpool.tile([P, T], F32R, tag=f"kt{m}", name=f"kt{m}") for m in range(NPAIR)]
            vp = vpool.tile([P, NKC, HG, DH + 1], F16, tag="vp")

            # ---------------- Phase 1: projections ----------------
            with (
                tc.tile_pool(name="xt", bufs=8) as xtpool,
                tc.tile_pool(name="wqks", bufs=16) as wqkpool,
                tc.tile_pool(name="wvs", bufs=1) as wvpool,
                tc.tile_pool(name="pproj", bufs=3, space="PSUM") as pproj,
                tc.tile_pool(name="pv", bufs=2, space="PSUM") as pvproj,
            ):
                xts = []
                for c in range(CCH):
                    xtile = xtpool.tile([P, T], F32R, tag="xt")
                    eng = nc.gpsimd if c % 2 else nc.sync
                    eng.dma_start(xtile[:], xt_v[c])
                    xts.append(xtile)
                wv_sb = wvpool.tile([P, CCH, 512], F32R, tag="wv")
                nc.gpsimd.dma_start(wv_sb[:], wv_v.rearrange("c p v -> p c v"))

                # QK projections: f 0..3 -> Q pairs, 4..7 -> K pairs
                for f in range(8):
                    wts = []
                    for c in range(CCH):
                        wt = wqkpool.tile([P, P], F32R, tag="wqk")
                        nc.sync.dma_start(wt[:], wqk_v[c, :, f, :])
                        wts.append(wt)
                    dst = qt[f] if f < 4 else kt[f - 4]
                    for t in range(NJ):
                        ps = pproj.tile([P, TB], F32, tag="pqk")
                        for c in range(CCH):
                            nc.tensor.matmul(
                                ps[:], wts[c][:], xts[c][:, _ts(t, TB)],
                                start=(c == 0), stop=(c == CCH - 1),
                            )
                        nc.vector.tensor_scalar(
                            dst[:, _ts(t, TB)], ps[:],
                            bqk_sb[:, f : f + 1], None, ALU.add,
                        )

                # V projection (all 8 heads at once), + bias via ones-matmul
                for tb in range(NKC):
                    ps = pvproj.tile([P, 512], F32, tag="pvp")
                    for c in range(CCH):
                        nc.tensor.matmul(
                            ps[:], xts[c][:, _ts(tb, P)], wv_sb[:, c],
                            start=(c == 0), stop=False,
                        )
                    nc.tensor.matmul(
                        ps[:], ones_sb[0:1, :], bv_sb[:], start=False, stop=True
                    )
                    for h in range(HG):
                        nc.vector.tensor_copy(
                            vp[:, tb, h, 0:DH], ps[:, _ts(h, DH)]
                        )
                # ones column of V' (DVE strided copy; a DMA would emit 16K 4B packets)
                nc.vector.tensor_copy(
                    vp[:, :, :, DH],
                    ones_sb[:, 0:P].rearrange("p (a b) -> p a b", a=NKC),
                )

            # ---------------- Phase 2+3: attention + out-proj ----------------
            with (
                tc.tile_pool(name="opair", bufs=1) as opool,
                tc.tile_pool(name="wos", bufs=1) as wopool,
                tc.tile_pool(name="pp", bufs=4) as ppool,
                tc.tile_pool(name="work", bufs=3) as work,
                tc.tile_pool(name="psS", bufs=2, space="PSUM") as psS,
                tc.tile_pool(name="psO", bufs=1, space="PSUM") as psO,
                tc.tile_pool(name="psM", bufs=1, space="PSUM") as psM,
            ):
                opair = [opool.tile([P, T], F32R, tag=f"op{m}", name=f"op{m}") for m in range(NPAIR)]
                wo_sb = wopool.tile([P, NPAIR, C], F32R, tag="wo")
                nc.gpsimd.dma_start(wo_sb[:], wo_v.rearrange("m p o -> p m o"))

                for J in range(NJ):
                    nch = 4 * J + 4
                    nmac = nch // 2
                    for m in range(NPAIR):
                        pso = [
                            psO.tile([DH + 1, TB], F32, tag=f"o{hl}", name=f"o{hl}")
                            for hl in range(2)
                        ]
                        def emit_pv(i2, c0s, pts):
                            for hl in range(2):
                                for sx in range(2):
                                    i = 2 * i2 + sx
                                    nc.tensor.matmul(
                                        pso[hl][:, c0s[sx]:TB],
                                        vp[:, i, 2 * m + hl, :],
                                        pts[hl][:, sx, c0s[sx]:TB],
                                        start=(i == 0), stop=(i == nch - 1),
                                    )
                            for hl in range(2):
                                if c0s[1] == 0:
                                    nc.sync.dma_start(
                                        attn_d[2 * m + hl, i2, J],
                                        pts[hl][:],
                                    )
                                else:
                                    for sx in range(2):
                                        nc.sync.dma_start(
                                            attn_d[2 * m + hl, i2, J, :, sx, c0s[sx]:TB],
                                            pts[hl][:, sx, c0s[sx]:TB],
                                        )

                        prev = None
                        for i2 in range(nmac):
                            c0s = [
                                P * (i - 4 * J) if i >= 4 * J else 0
                                for i in (2 * i2, 2 * i2 + 1)
                            ]
                            pss = []
                            pts = []
                            for hl in range(2):
                                pr = 64 * hl
                                ps = psS.tile([P, 2, TB], F32, tag="S", name=f"S{hl}")
                                # adjacent K=64 matmuls on row groups 0/64
                                for sx in range(2):
                                    i = 2 * i2 + sx
                                    nc.tensor.matmul(
                                        ps[:, sx, c0s[sx]:TB],
                                        kt[m][pr : pr + 64, _ts(i, P)],
                                        qt[m][pr : pr + 64, TB * J + c0s[sx] : TB * (J + 1)],
                                        start=True, stop=True,
                                    )
                                pss.append(ps)
                            for hl in range(2):
                                pt = ppool.tile([P, 2, TB], F16, tag=f"P{hl}", name=f"P{hl}")
                                c0a = c0s[0]
                                nc.scalar.activation(
                                    pt[:, :, c0a:TB], pss[hl][:, :, c0a:TB],
                                    AF.Exp, scale=0.125,
                                )
                                for sx in range(2):
                                    i = 2 * i2 + sx
                                    if i >= 4 * J:
                                        c0 = c0s[sx]
                                        nc.vector.tensor_tensor(
                                            pt[:, sx, c0 : c0 + P],
                                            pt[:, sx, c0 : c0 + P],
                                            tri_sb[:], ALU.mult,
                                        )
                                pts.append(pt)
                            if prev is not None:
                                emit_pv(*prev)
                            prev = (i2, c0s, pts)
                        emit_pv(*prev)
                        for hl in range(2):
                            h = 2 * m + hl
                            pr = 64 * hl
                            # free pso quickly: only the two copies read it
                            ssb = work.tile([1, TB], F32, tag="ssb")
                            nc.vector.tensor_copy(ssb[:], pso[hl][DH : DH + 1, :])
                            osb = work.tile([64, TB], F32, tag="osb")
                            nc.vector.tensor_copy(osb[:], pso[hl][0:DH, :])
                            nc.sync.dma_start(sums_d[h, _ts(J, 1), :], ssb[:])
                            # fast reciprocal (~18 bits, enough before f32r round)
                            r0 = work.tile([1, TB], F32, tag="r0")
                            nc.vector.reciprocal_approx_fast(r0[:], ssb[:])
                            r1 = work.tile([1, TB], F32R, tag="r1")
                            nc.vector.tensor_copy(r1[:], r0[:])
                            # broadcast recip across 64 partitions
                            pb = psM.tile([64, TB], F32, tag="b")
                            nc.tensor.matmul(
                                pb[:], ones_sb[0:1, 0:64], r1[:],
                                start=True, stop=True,
                            )
                            nc.vector.tensor_tensor(
                                opair[m][pr : pr + 64, _ts(J, TB)],
                                osb[:], pb[:], ALU.mult,
                            )
                    # out-projection for this q-block
                    for ob in range(8):
                        py = psM.tile([P, TB], F32, tag="y")
                        for m in range(NPAIR):
                            nc.tensor.matmul(
                                py[:],
                                wo_sb[:, m, _ts(ob, P)],
                                opair[m][:, _ts(J, TB)],
                                start=(m == 0), stop=(m == NPAIR - 1),
                            )
                        ysb = work.tile([P, TB], F32, tag="ysb")
                        nc.scalar.copy(ysb[:], py[:])
                        nc.sync.dma_start(yt_v[ob][:, _ts(J, TB)], ysb[:])

    nc.finalize()
    return nc


def _get_module():
    if "nc" not in _NC_CACHE:
        _NC_CACHE["nc"] = _build_module()
    return _NC_CACHE["nc"]


def kernel(x, qkv_w, qkv_b, out_w, out_b):
    global LAST_EXEC_NS
    x = np.asarray(x, dtype=np.float32)
    qkv_w = np.asarray(qkv_w, dtype=np.float32)
    qkv_b = np.asarray(qkv_b, dtype=np.float32)
    out_w = np.asarray(out_w, dtype=np.float32)
    out_b = np.asarray(out_b, dtype=np.float32)

    qw, kw, vw = qkv_w[0:C], qkv_w[C : 2 * C], qkv_w[2 * C : 3 * C]
    tri = np.triu(np.ones((P, P), dtype=np.float16))
    ones = np.ones((P, P), dtype=np.float32)

    in_maps = []
    for core in range(8):
        b, g = core // 2, core % 2
        sl = slice(512 * g, 512 * (g + 1))
        wqk = np.concatenate([qw[sl], kw[sl]], axis=0).T  # [C, 1024]
        bqk = np.concatenate(
            [qkv_b[C * 0 + 512 * g : C * 0 + 512 * (g + 1)],
             qkv_b[C * 1 + 512 * g : C * 1 + 512 * (g + 1)]]
        ).reshape(8, P).T  # [128, 8] column f
        in_maps.append({
            "xt": _round_f32r(x[b].T),
            "wqk": _round_f32r(wqk),
            "wv": _round_f32r(vw[sl].T),
            "wo": _round_f32r(out_w[:, sl].T),
            "bqk": np.ascontiguousarray(bqk),
            "bv": _round_f32r(qkv_b[2 * C + 512 * g : 2 * C + 512 * (g + 1)][None, :]),
            "tri": tri,
            "ones": ones,
        })

    nc = _get_module()
    res = run_bass_kernel_spmd(
        nc, in_maps, core_ids=list(range(8)), trace=TRACE,
    )
    LAST_EXEC_NS = res.exec_time_ns

    out = np.empty((B, T, C), dtype=np.float32)
    attn = np.empty((B, H, T, T), dtype=np.float32)
    for b in range(B):
        y0 = res.results[2 * b]["yt"]
        y1 = res.results[2 * b + 1]["yt"]
        out[b] = y0.T + y1.T + out_b[None, :]
    for core in range(8):
        b, g = core // 2, core % 2
        shard = res.results[core]["attn_t"]  # [HG, NKC, NJ, P, TB] fp16 (k,q blocked)
        sums = res.results[core]["sums"].reshape(HG, T)
        for h in range(HG):
            recip = (1.0 / sums[h]).astype(np.float32)
            blk = shard[h].transpose(1, 4, 0, 3, 2).reshape(T, T)  # -> [q, k]
            np.multiply(blk, recip[:, None], out=attn[b, 8 * g + h])
    return out, attn


# revision 19
# speedup vs baseline: 1.0411x; 1.0411x over previous
"""Causal self-attention Trainium2 kernel (B=4, T=2048, C=1024, H=16, Dh=64).

Sharding: 8 cores = (batch b) x (head-group g of 8 heads).
Per-core: transposed flash attention in float32r (see DESIGN.md).
Returns (out [B,T,C] f32, attn_weights [B,H,T,T] f32) like the reference.
"""

import sys

sys.path.insert(0, "/opt/trn_rl_repo")

import numpy as np

import concourse.bass as bass  # noqa: F401
import concourse.mybir as mybir
from concourse import bacc
from concourse.bass_utils import run_bass_kernel_spmd
from concourse.tile import TileContext

F32 = mybir.dt.float32
F32R = mybir.dt.float32r
F16 = mybir.dt.float16
AF = mybir.ActivationFunctionType
ALU = mybir.AluOpType

B, T, C = 4, 2048, 1024
H, DH = 16, 64
HG = 8          # heads per core
NPAIR = 4       # head pairs per core
P = 128
TB = 512        # q-block width
NJ = T // TB    # 4 q-blocks
NKC = T // P    # 16 k-chunks
CCH = C // P    # 8 contraction chunks

TRACE = False
LAST_EXEC_NS = None
_NC_CACHE = {}


def _round_f32r(x):
    """Round fp32 to 13-bit mantissa (round-to-nearest-even), matching DVE."""
    b = np.ascontiguousarray(x, dtype=np.float32).view(np.uint32)
    lsb = (b >> 10) & np.uint32(1)
    b = b + np.uint32(0x1FF) + lsb
    b &= np.uint32(0xFFFFFC00)
    return b.view(np.float32)


def _ts(i, s):
    return slice(i * s, (i + 1) * s)


def _build_module():
    nc = bacc.Bacc("TRN2")

    xt_d = nc.dram_tensor("xt", [C, T], F32R, kind="ExternalInput")
    wqk_d = nc.dram_tensor("wqk", [C, 1024], F32R, kind="ExternalInput")
    wv_d = nc.dram_tensor("wv", [C, 512], F32R, kind="ExternalInput")
    wo_d = nc.dram_tensor("wo", [512, C], F32R, kind="ExternalInput")
    bqk_d = nc.dram_tensor("bqk", [P, 8], F32, kind="ExternalInput")
    bv_d = nc.dram_tensor("bv", [1, 512], F32R, kind="ExternalInput")
    tri_d = nc.dram_tensor("tri", [P, P], F16, kind="ExternalInput")
    ones_d = nc.dram_tensor("ones", [P, P], F32R, kind="ExternalInput")

    attn_d = nc.dram_tensor("attn_t", [HG, NKC // 2, NJ, P, 2, TB], F16, kind="ExternalOutput")
    yt_d = nc.dram_tensor("yt", [C, T], F32, kind="ExternalOutput")
    sums_d = nc.dram_tensor("sums", [HG, NJ, TB], F32, kind="ExternalOutput")

    xt_v = xt_d.rearrange("(c p) t -> c p t", p=P)          # [8,128,2048]
    wqk_v = wqk_d.rearrange("(c p) (f j) -> c p f j", p=P, j=P)  # [8,128,8,128]
    wv_v = wv_d.rearrange("(c p) v -> c p v", p=P)          # [8,128,512]
    wo_v = wo_d.rearrange("(m p) o -> m p o", p=P)          # [4,128,1024]
    yt_v = yt_d.rearrange("(o p) t -> o p t", p=P)          # [8,128,2048]

    with TileContext(nc) as tc, nc.allow_low_precision(reason="fp32r attention"):
        import contextlib

        with contextlib.ExitStack() as ctx:
            cpool = ctx.enter_context(tc.tile_pool(name="consts", bufs=1))
            qkpool = ctx.enter_context(tc.tile_pool(name="qk", bufs=1))
            vpool = ctx.enter_context(tc.tile_pool(name="vp", bufs=1))

            tri_sb = cpool.tile([P, P], F16, tag="tri")
            ones_sb = cpool.tile([P, P], F32R, tag="ones")
            bqk_sb = cpool.tile([P, 8], F32, tag="bqk")
            bv_sb = cpool.tile([1, 512], F32R, tag="bv")
            nc.sync.dma_start(tri_sb[:], tri_d[:])
            nc.sync.dma_start(ones_sb[:], ones_d[:])
            nc.sync.dma_start(bqk_sb[:], bqk_d[:])
            nc.sync.dma_start(bv_sb[:], bv_d[:])

            # Persistent activation tiles
            qt = [qkpool.tile([P, T], F32R, tag=f"qt{m}", name=f"qt{m}") for m in range(NPAIR)]
            kt = [qkpool.tile([P, T], F32R, tag=f"kt{m}", name=f"kt{m}") for m in range(NPAIR)]
            vp = vpool.tile([P, NKC, HG, DH + 1], F16, tag="vp")

            # ---------------- Phase 1: projections ----------------
            with (
                tc.tile_pool(name="xt", bufs=8) as xtpool,
                tc.tile_pool(name="wqks", bufs=16) as wqkpool,
                tc.tile_pool(name="wvs", bufs=1) as wvpool,
                tc.tile_pool(name="pproj", bufs=4, space="PSUM") as pproj,
                tc.tile_pool(name="pv", bufs=2, space="PSUM") as pvproj,
            ):
                xts = []
                for c in range(CCH):
                    xtile = xtpool.tile([P, T], F32R, tag="xt")
                    eng = nc.gpsimd if c % 2 else nc.sync
                    eng.dma_start(xtile[:], xt_v[c])
                    xts.append(xtile)
                wv_sb = wvpool.tile([P, CCH, 512], F32R, tag="wv")
                nc.gpsimd.dma_start(wv_sb[:], wv_v.rearrange("c p v -> p c v"))

                # QK projections: f 0..3 -> Q pairs, 4..7 -> K pairs
                for f in range(8):
                    wts = []
                    for c in range(CCH):
                        wt = wqkpool.tile([P, P], F32R, tag="wqk")
                        nc.sync.dma_start(wt[:], wqk_v[c, :, f, :])
                        wts.append(wt)
                    dst = qt[f] if f < 4 else kt[f - 4]
                    for t in range(NJ):
                        ps = pproj.tile([P, TB], F32, tag="pqk")
                        for c in range(CCH):
                            nc.tensor.matmul(
                                ps[:], wts[c][:], xts[c][:, _ts(t, TB)],
                                start=(c == 0), stop=(c == CCH - 1),
                            )
                        nc.vector.tensor_scalar(
                            dst[:, _ts(t, TB)], ps[:],
                            bqk_sb[:, f : f + 1], None, ALU.add,
                        )

                # V projection (all 8 heads at once), + bias via ones-matmul
                for tb in range(NKC):
                    ps = pvproj.tile([P, 512], F32, tag="pvp")
                    for c in range(CCH):
                        nc.tensor.matmul(
                            ps[:], xts[c][:, _ts(tb, P)], wv_sb[:, c],
                            start=(c == 0), stop=False,
                        )
                    nc.tensor.matmul(
                        ps[:], ones_sb[0:1, :], bv_sb[:], start=False, stop=True
                    )
                    for h in range(HG):
                        nc.vector.tensor_copy(
                            vp[:, tb, h, 0:DH], ps[:, _ts(h, DH)]
                        )
                # ones column of V' (DVE strided copy; a DMA would emit 16K 4B packets)
                nc.vector.tensor_copy(
                    vp[:, :, :, DH],
                    ones_sb[:, 0:P].rearrange("p (a b) -> p a b", a=NKC),
                )

            # ---------------- Phase 2+3: attention + out-proj ----------------
            with (
                tc.tile_pool(name="opair", bufs=1) as opool,
                tc.tile_pool(name="wos", bufs=1) as wopool,
                tc.tile_pool(name="pp", bufs=4) as ppool,
                tc.tile_pool(name="work", bufs=3) as work,
                tc.tile_pool(name="psS", bufs=2, space="PSUM") as psS,
                tc.tile_pool(name="psO", bufs=1, space="PSUM") as psO,
                tc.tile_pool(name="psM", bufs=1, space="PSUM") as psM,
            ):
                opair = [opool.tile([P, T], F32R, tag=f"op{m}", name=f"op{m}") for m in range(NPAIR)]
                wo_sb = wopool.tile([P, NPAIR, C], F32R, tag="wo")
                nc.gpsimd.dma_start(wo_sb[:], wo_v.rearrange("m p o -> p m o"))

                for J in range(NJ):
                    nch = 4 * J + 4
                    nmac = nch // 2
                    for m in range(NPAIR):
                        pso = [
                            psO.tile([DH + 1, TB], F32, tag=f"o{hl}", name=f"o{hl}")
                            for hl in range(2)
                        ]
                        def emit_pv(i2, c0s, pts):
                            for hl in range(2):
                                for sx in range(2):
                                    i = 2 * i2 + sx
                                    nc.tensor.matmul(
                                        pso[hl][:, c0s[sx]:TB],
                                        vp[:, i, 2 * m + hl, :],
                                        pts[hl][:, sx, c0s[sx]:TB],
                                        start=(i == 0), stop=(i == nch - 1),
                                    )
                            for hl in range(2):
                                if c0s[1] == 0:
                                    nc.sync.dma_start(
                                        attn_d[2 * m + hl, i2, J],
                                        pts[hl][:],
                                    )
                                else:
                                    for sx in range(2):
                                        nc.sync.dma_start(
                                            attn_d[2 * m + hl, i2, J, :, sx, c0s[sx]:TB],
                                            pts[hl][:, sx, c0s[sx]:TB],
                                        )

                        prev = None
                        for i2 in range(nmac):
                            c0s = [
                                P * (i - 4 * J) if i >= 4 * J else 0
                                for i in (2 * i2, 2 * i2 + 1)
                            ]
                            pss = []
                            pts = []
                            for hl in range(2):
                                pr = 64 * hl
                                ps = psS.tile([P, 2, TB], F32, tag="S", name=f"S{hl}")
                                # adjacent K=64 matmuls on row groups 0/64
                                for sx in range(2):
                                    i = 2 * i2 + sx
                                    nc.tensor.matmul(
                                        ps[:, sx, c0s[sx]:TB],
                                        kt[m][pr : pr + 64, _ts(i, P)],
                                        qt[m][pr : pr + 64, TB * J + c0s[sx] : TB * (J + 1)],
                                        start=True, stop=True,
                                    )
                                pss.append(ps)
                            for hl in range(2):
                                pt = ppool.tile([P, 2, TB], F16, tag=f"P{hl}", name=f"P{hl}")
                                c0a = c0s[0]
                                nc.scalar.activation(
                                    pt[:, :, c0a:TB], pss[hl][:, :, c0a:TB],
                                    AF.Exp, scale=0.125,
                                )
                                for sx in range(2):
                                    i = 2 * i2 + sx
                                    if i >= 4 * J:
                                        c0 = c0s[sx]
                                        nc.vector.tensor_tensor(
                                            pt[:, sx, c0 : c0 + P],
                                            pt[:, sx, c0 : c0 + P],
                                            tri_sb[:], ALU.mult,
                                        )
                                pts.append(pt)
                            if prev is not None:
                                emit_pv(*prev)
                            prev = (i2, c0s, pts)
                        emit_pv(*prev)
                        for hl in range(2):
                            h = 2 * m + hl
                            pr = 64 * hl
                            # free pso quickly: only the two copies read it
                            ssb = work.tile([1, TB], F32, tag="ssb")
                            nc.vector.tensor_copy(ssb[:], pso[hl][DH : DH + 1, :])
                            osb = work.tile([64, TB], F32, tag="osb")
                            nc.vector.tensor_copy(osb[:], pso[hl][0:DH, :])
                            nc.sync.dma_start(sums_d[h, _ts(J, 1), :], ssb[:])
                            # fast reciprocal (~18 bits, enough before f32r round)
                            r0 = work.tile([1, TB], F32, tag="r0")
                            nc.vector.reciprocal_approx_fast(r0[:], ssb[:])
                            r1 = work.tile([1, TB], F32R, tag="r1")
                            nc.vector.tensor_copy(r1[:], r0[:])
                            # broadcast recip across 64 partitions
                            pb = psM.tile([64, TB], F32, tag="b")
                            nc.tensor.matmul(
                                pb[:], ones_sb[0:1, 0:64], r1[:],
                                start=True, stop=True,
                            )
                            nc.vector.tensor_tensor(
                                opair[m][pr : pr + 64, _ts(J, TB)],
                                osb[:], pb[:], ALU.mult,
                            )
                    # out-projection for this q-block
                    for ob in range(8):
                        py = psM.tile([P, TB], F32, tag="y")
                        for m in range(NPAIR):
                            nc.tensor.matmul(
                                py[:],
                                wo_sb[:, m, _ts(ob, P)],
                                opair[m][:, _ts(J, TB)],
                                start=(m == 0), stop=(m == NPAIR - 1),
                            )
                        ysb = work.tile([P, TB], F32, tag="ysb")
                        nc.scalar.copy(ysb[:], py[:])
                        nc.sync.dma_start(yt_v[ob][:, _ts(J, TB)], ysb[:])

    nc.finalize()
    return nc


def _get_module():
    if "nc" not in _NC_CACHE:
        _NC_CACHE["nc"] = _build_module()
    return _NC_CACHE["nc"]


def kernel(x, qkv_w, qkv_b, out_w, out_b):
    global LAST_EXEC_NS
    x = np.asarray(x, dtype=np.float32)
    qkv_w = np.asarray(qkv_w, dtype=np.float32)
    qkv_b = np.asarray(qkv_b, dtype=np.float32)
    out_w = np.asarray(out_w, dtype=np.float32)
    out_b = np.asarray(out_b, dtype=np.float32)

    qw, kw, vw = qkv_w[0:C], qkv_w[C : 2 * C], qkv_w[2 * C : 3 * C]
    tri = np.triu(np.ones((P, P), dtype=np.float16))
    ones = np.ones((P, P), dtype=np.float32)

    in_maps = []
    for core in range(8):
        b, g = core // 2, core % 2
        sl = slice(512 * g, 512 * (g + 1))
        wqk = np.concatenate([qw[sl], kw[sl]], axis=0).T  # [C, 1024]
        bqk = np.concatenate(
            [qkv_b[C * 0 + 512 * g : C * 0 + 512 * (g + 1)],
             qkv_b[C * 1 + 512 * g : C * 1 + 512 * (g + 1)]]
        ).reshape(8, P).T  # [128, 8] column f
        in_maps.append({
            "xt": _round_f32r(x[b].T),
            "wqk": _round_f32r(wqk),
            "wv": _round_f32r(vw[sl].T),
            "wo": _round_f32r(out_w[:, sl].T),
            "bqk": np.ascontiguousarray(bqk),
            "bv": _round_f32r(qkv_b[2 * C + 512 * g : 2 * C + 512 * (g + 1)][None, :]),
            "tri": tri,
            "ones": ones,
        })

    nc = _get_module()
    res = run_bass_kernel_spmd(
        nc, in_maps, core_ids=list(range(8)), trace=TRACE,
    )
    LAST_EXEC_NS = res.exec_time_ns

    out = np.empty((B, T, C), dtype=np.float32)
    attn = np.empty((B, H, T, T), dtype=np.float32)
    for b in range(B):
        y0 = res.results[2 * b]["yt"]
        y1 = res.results[2 * b + 1]["yt"]
        out[b] = y0.T + y1.T + out_b[None, :]
    for core in range(8):
        b, g = core // 2, core % 2
        shard = res.results[core]["attn_t"]  # [HG, NKC, NJ, P, TB] fp16 (k,q blocked)
        sums = res.results[core]["sums"].reshape(HG, T)
        for h in range(HG):
            recip = (1.0 / sums[h]).astype(np.float32)
            blk = shard[h].transpose(1, 4, 0, 3, 2).reshape(T, T)  # -> [q, k]
            np.multiply(blk, recip[:, None], out=attn[b, 8 * g + h])
    return out, attn


# revision 23
# speedup vs baseline: 1.1073x; 1.0636x over previous
"""Causal self-attention Trainium2 kernel (B=4, T=2048, C=1024, H=16, Dh=64).

Sharding: 8 cores = (batch b) x (head-group g of 8 heads).
Per-core: transposed flash attention in float32r (see DESIGN.md).
Returns (out [B,T,C] f32, attn_weights [B,H,T,T] f32) like the reference.
"""

import sys

sys.path.insert(0, "/opt/trn_rl_repo")

import numpy as np

import concourse.bass as bass  # noqa: F401
import concourse.mybir as mybir
from concourse import bacc
from concourse.bass_utils import run_bass_kernel_spmd
from concourse.tile import TileContext

F32 = mybir.dt.float32
F32R = mybir.dt.float32r
F16 = mybir.dt.float16
AF = mybir.ActivationFunctionType
ALU = mybir.AluOpType

B, T, C = 4, 2048, 1024
H, DH = 16, 64
HG = 8          # heads per core
NPAIR = 4       # head pairs per core
P = 128
TB = 512        # q-block width
NJ = T // TB    # 4 q-blocks
NKC = T // P    # 16 k-chunks
CCH = C // P    # 8 contraction chunks

TRACE = False
LAST_EXEC_NS = None
_NC_CACHE = {}


def _round_f32r(x):
    """Round fp32 to 13-bit mantissa (round-to-nearest-even), matching DVE."""
    b = np.ascontiguousarray(x, dtype=np.float32).view(np.uint32)
    lsb = (b >> 10) & np.uint32(1)
    b = b + np.uint32(0x1FF) + lsb
    b &= np.uint32(0xFFFFFC00)
    return b.view(np.float32)


def _ts(i, s):
    return slice(i * s, (i + 1) * s)


def _build_module():
    nc = bacc.Bacc("TRN2")

    xt_d = nc.dram_tensor("xt", [C, T], F32R, kind="ExternalInput")
    wqk_d = nc.dram_tensor("wqk", [C, 1024], F32R, kind="ExternalInput")
    wv_d = nc.dram_tensor("wv", [C, 512], F32R, kind="ExternalInput")
    wo_d = nc.dram_tensor("wo", [512, C], F32R, kind="ExternalInput")
    bqk_d = nc.dram_tensor("bqk", [P, 8], F32, kind="ExternalInput")
    bv_d = nc.dram_tensor("bv", [1, 512], F32R, kind="ExternalInput")
    tri_d = nc.dram_tensor("tri", [P, P], F16, kind="ExternalInput")
    ones_d = nc.dram_tensor("ones", [P, P], F32R, kind="ExternalInput")

    attn_d = nc.dram_tensor("attn_t", [HG, NKC // 2, NJ, P, 2, TB], F16, kind="ExternalOutput")
    yt_d = nc.dram_tensor("yt", [C, T], F32, kind="ExternalOutput")
    sums_d = nc.dram_tensor("sums", [HG, NJ, TB], F32, kind="ExternalOutput")

    xt_v = xt_d.rearrange("(c p) t -> c p t", p=P)          # [8,128,2048]
    wqk_v = wqk_d.rearrange("(c p) (f j) -> c p f j", p=P, j=P)  # [8,128,8,128]
    wv_v = wv_d.rearrange("(c p) v -> c p v", p=P)          # [8,128,512]
    wo_v = wo_d.rearrange("(m p) o -> m p o", p=P)          # [4,128,1024]
    yt_v = yt_d.rearrange("(o p) t -> o p t", p=P)          # [8,128,2048]

    with TileContext(nc) as tc, nc.allow_low_precision(reason="fp32r attention"):
        import contextlib

        with contextlib.ExitStack() as ctx:
            cpool = ctx.enter_context(tc.tile_pool(name="consts", bufs=1))
            qkpool = ctx.enter_context(tc.tile_pool(name="qk", bufs=1))
            vpool = ctx.enter_context(tc.tile_pool(name="vp", bufs=1))

            tri_sb = cpool.tile([P, P], F16, tag="tri")
            ones_sb = cpool.tile([P, P], F32R, tag="ones")
            bqk_sb = cpool.tile([P, 8], F32, tag="bqk")
            bv_sb = cpool.tile([1, 512], F32R, tag="bv")
            nc.sync.dma_start(tri_sb[:], tri_d[:])
            nc.sync.dma_start(ones_sb[:], ones_d[:])
            nc.sync.dma_start(bqk_sb[:], bqk_d[:])
            nc.sync.dma_start(bv_sb[:], bv_d[:])

            # Persistent activation tiles
            qt = [qkpool.tile([P, T], F32R, tag=f"qt{m}", name=f"qt{m}") for m in range(NPAIR)]
            kt = [qkpool.tile([P, T], F32R, tag=f"kt{m}", name=f"kt{m}") for m in range(NPAIR)]
            vp = vpool.tile([P, NKC, HG, DH + 1], F16, tag="vp")

            # ---------------- Phase 1: projections ----------------
            with (
                tc.tile_pool(name="xt", bufs=8) as xtpool,
                tc.tile_pool(name="wqks", bufs=16) as wqkpool,
                tc.tile_pool(name="wvs", bufs=1) as wvpool,
                tc.tile_pool(name="pproj", bufs=4, space="PSUM") as pproj,
                tc.tile_pool(name="pv", bufs=2, space="PSUM") as pvproj,
            ):
                xts = []
                for c in range(CCH):
                    xtile = xtpool.tile([P, T], F32R, tag="xt")
                    nc.gpsimd.dma_start(xtile[:], xt_v[c])
                    xts.append(xtile)
                wv_sb = wvpool.tile([P, CCH, 512], F32R, tag="wv")
                nc.gpsimd.dma_start(wv_sb[:], wv_v.rearrange("c p v -> p c v"))

                # QK projections: f 0..3 -> Q pairs, 4..7 -> K pairs
                for f in range(8):
                    wts = []
                    for c in range(CCH):
                        wt = wqkpool.tile([P, P], F32R, tag="wqk")
                        nc.sync.dma_start(wt[:], wqk_v[c, :, f, :])
                        wts.append(wt)
                    dst = qt[f] if f < 4 else kt[f - 4]
                    for t in range(NJ):
                        ps = pproj.tile([P, TB], F32, tag="pqk")
                        for c in range(CCH):
                            nc.tensor.matmul(
                                ps[:], wts[c][:], xts[c][:, _ts(t, TB)],
                                start=(c == 0), stop=(c == CCH - 1),
                            )
                        nc.vector.tensor_scalar(
                            dst[:, _ts(t, TB)], ps[:],
                            bqk_sb[:, f : f + 1], None, ALU.add,
                        )

                # V projection (all 8 heads at once), + bias via ones-matmul
                for tb in range(NKC):
                    ps = pvproj.tile([P, 512], F32, tag="pvp")
                    for c in range(CCH):
                        nc.tensor.matmul(
                            ps[:], xts[c][:, _ts(tb, P)], wv_sb[:, c],
                            start=(c == 0), stop=False,
                        )
                    nc.tensor.matmul(
                        ps[:], ones_sb[0:1, :], bv_sb[:], start=False, stop=True
                    )
                    for h in range(HG):
                        nc.vector.tensor_copy(
                            vp[:, tb, h, 0:DH], ps[:, _ts(h, DH)]
                        )
                # ones column of V' (DVE strided copy; a DMA would emit 16K 4B packets)
                nc.vector.tensor_copy(
                    vp[:, :, :, DH],
                    ones_sb[:, 0:P].rearrange("p (a b) -> p a b", a=NKC),
                )

            # ---------------- Phase 2+3: attention + out-proj ----------------
            with (
                tc.tile_pool(name="opair", bufs=1) as opool,
                tc.tile_pool(name="wos", bufs=1) as wopool,
                tc.tile_pool(name="pp", bufs=3) as ppool,
                tc.tile_pool(name="work", bufs=3) as work,
                tc.tile_pool(name="psS", bufs=2, space="PSUM") as psS,
                tc.tile_pool(name="psO", bufs=1, space="PSUM") as psO,
            ):
                opair = [opool.tile([P, T], F32R, tag=f"op{m}", name=f"op{m}") for m in range(NPAIR)]
                wo_sb = wopool.tile([P, NPAIR, C], F32R, tag="wo")
                nc.gpsimd.dma_start(wo_sb[:], wo_v.rearrange("m p o -> p m o"))

                for J in range(NJ):
                    nch = 4 * J + 4
                    nmac = nch // 2

                    def emit_s_exp(m, i2, c0s, pi):
                        pss = []
                        pts = []
                        for hl in range(2):
                            pr = 64 * hl
                            ps = psS.tile([P, 2, TB], F32, tag="S", name=f"S{pi}{hl}")
                            # adjacent K=64 matmuls on row groups 0/64
                            for sx in range(2):
                                i = 2 * i2 + sx
                                nc.tensor.matmul(
                                    ps[:, sx, c0s[sx]:TB],
                                    kt[m][pr : pr + 64, _ts(i, P)],
                                    qt[m][pr : pr + 64, TB * J + c0s[sx] : TB * (J + 1)],
                                    start=True, stop=True,
                                )
                            pss.append(ps)
                        for hl in range(2):
                            pt = ppool.tile(
                                [P, 2, TB], F16, tag=f"P{pi}{hl}", name=f"P{pi}{hl}"
                            )
                            if c0s[1] == 0:
                                # one exp over both k-chunks (1024 free)
                                nc.scalar.activation(
                                    pt[:], pss[hl][:], AF.Exp, scale=0.125
                                )
                            else:
                                for sx in range(2):
                                    c0 = c0s[sx]
                                    nc.scalar.activation(
                                        pt[:, sx, c0:TB], pss[hl][:, sx, c0:TB],
                                        AF.Exp, scale=0.125,
                                    )
                            for sx in range(2):
                                i = 2 * i2 + sx
                                if i >= 4 * J:
                                    c0 = c0s[sx]
                                    nc.vector.tensor_tensor(
                                        pt[:, sx, c0 : c0 + P],
                                        pt[:, sx, c0 : c0 + P],
                                        tri_sb[:], ALU.mult,
                                    )
                            pts.append(pt)
                        return pts

                    def emit_pv(m, pso, i2, c0s, pts):
                        for hl in range(2):
                            for sx in range(2):
                                i = 2 * i2 + sx
                                nc.tensor.matmul(
                                    pso[hl][:, c0s[sx]:TB],
                                    vp[:, i, 2 * m + hl, :],
                                    pts[hl][:, sx, c0s[sx]:TB],
                                    start=(i == 0), stop=(i == nch - 1),
                                )
                        for hl in range(2):
                            if c0s[1] == 0:
                                nc.sync.dma_start(
                                    attn_d[2 * m + hl, i2, J],
                                    pts[hl][:],
                                )
                            else:
                                for sx in range(2):
                                    nc.sync.dma_start(
                                        attn_d[2 * m + hl, i2, J, :, sx, c0s[sx]:TB],
                                        pts[hl][:, sx, c0s[sx]:TB],
                                    )

                    def emit_tail(m, pso, pi):
                        for hl in range(2):
                            h = 2 * m + hl
                            pr = 64 * hl
                            # free pso quickly: only the two copies read it
                            ssb = work.tile([1, TB], F32, tag="ssb")
                            nc.vector.tensor_copy(ssb[:], pso[hl][DH : DH + 1, :])
                            osb = work.tile([64, TB], F32, tag="osb")
                            nc.vector.tensor_copy(osb[:], pso[hl][0:DH, :])
                            nc.sync.dma_start(sums_d[h, _ts(J, 1), :], ssb[:])
                            # fast reciprocal (~18 bits, enough before f32r round)
                            r0 = work.tile([1, TB], F32, tag="r0")
                            nc.vector.reciprocal_approx_fast(r0[:], ssb[:])
                            r1 = work.tile([1, TB], F32R, tag="r1")
                            nc.vector.tensor_copy(r1[:], r0[:])
                            # broadcast recip across 64 partitions
                            pb = psO.tile(
                                [64, TB], F32, tag=f"o{pi}{hl}", name=f"b{pi}{hl}"
                            )
                            nc.tensor.matmul(
                                pb[:], ones_sb[0:1, 0:64], r1[:],
                                start=True, stop=True,
                            )
                            nc.vector.tensor_tensor(
                                opair[m][pr : pr + 64, _ts(J, TB)],
                                osb[:], pb[:], ALU.mult,
                            )

                    # two head-pairs interleaved for engine density
                    for mg in range(0, NPAIR, 2):
                        psos = {}
                        for pi in range(2):
                            psos[pi] = [
                                psO.tile(
                                    [DH + 1, TB], F32,
                                    tag=f"o{pi}{hl}", name=f"o{pi}{hl}",
                                )
                                for hl in range(2)
                            ]
                        prev = {0: None, 1: None}
                        for i2 in range(nmac):
                            c0s = [
                                P * (i - 4 * J) if i >= 4 * J else 0
                                for i in (2 * i2, 2 * i2 + 1)
                            ]
                            for pi in range(2):
                                m = mg + pi
                                pts = emit_s_exp(m, i2, c0s, pi)
                                if prev[pi] is not None:
                                    emit_pv(m, psos[pi], *prev[pi])
                                prev[pi] = (i2, c0s, pts)
                        for pi in range(2):
                            emit_pv(mg + pi, psos[pi], *prev[pi])
                        for pi in range(2):
                            emit_tail(mg + pi, psos[pi], pi)
                    # out-projection for this q-block
                    for ob in range(8):
                        py = psO.tile([P, TB], F32, tag=f"o{ob % 2}0", name="y")
                        for m in range(NPAIR):
                            nc.tensor.matmul(
                                py[:],
                                wo_sb[:, m, _ts(ob, P)],
                                opair[m][:, _ts(J, TB)],
                                start=(m == 0), stop=(m == NPAIR - 1),
                            )
                        ysb = work.tile([P, TB], F32, tag="ysb")
                        nc.scalar.copy(ysb[:], py[:])
                        nc.sync.dma_start(yt_v[ob][:, _ts(J, TB)], ysb[:])

    nc.finalize()
    return nc


def _get_module():
    if "nc" not in _NC_CACHE:
        _NC_CACHE["nc"] = _build_module()
    return _NC_CACHE["nc"]


def kernel(x, qkv_w, qkv_b, out_w, out_b):
    global LAST_EXEC_NS
    x = np.asarray(x, dtype=np.float32)
    qkv_w = np.asarray(qkv_w, dtype=np.float32)
    qkv_b = np.asarray(qkv_b, dtype=np.float32)
    out_w = np.asarray(out_w, dtype=np.float32)
    out_b = np.asarray(out_b, dtype=np.float32)

    qw, kw, vw = qkv_w[0:C], qkv_w[C : 2 * C], qkv_w[2 * C : 3 * C]
    tri = np.triu(np.ones((P, P), dtype=np.float16))
    ones = np.ones((P, P), dtype=np.float32)

    in_maps = []
    for core in range(8):
        b, g = core // 2, core % 2
        sl = slice(512 * g, 512 * (g + 1))
        wqk = np.concatenate([qw[sl], kw[sl]], axis=0).T  # [C, 1024]
        bqk = np.concatenate(
            [qkv_b[C * 0 + 512 * g : C * 0 + 512 * (g + 1)],
             qkv_b[C * 1 + 512 * g : C * 1 + 512 * (g + 1)]]
        ).reshape(8, P).T  # [128, 8] column f
        in_maps.append({
            "xt": _round_f32r(x[b].T),
            "wqk": _round_f32r(wqk),
            "wv": _round_f32r(vw[sl].T),
            "wo": _round_f32r(out_w[:, sl].T),
            "bqk": np.ascontiguousarray(bqk),
            "bv": _round_f32r(qkv_b[2 * C + 512 * g : 2 * C + 512 * (g + 1)][None, :]),
            "tri": tri,
            "ones": ones,
        })

    nc = _get_module()
    res = run_bass_kernel_spmd(
        nc, in_maps, core_ids=list(range(8)), trace=TRACE,
    )
    LAST_EXEC_NS = res.exec_time_ns

    out = np.empty((B, T, C), dtype=np.float32)
    attn = np.empty((B, H, T, T), dtype=np.float32)
    for b in range(B):
        y0 = res.results[2 * b]["yt"]
        y1 = res.results[2 * b + 1]["yt"]
        out[b] = y0.T + y1.T + out_b[None, :]
    for core in range(8):
        b, g = core // 2, core % 2
        shard = res.results[core]["attn_t"]  # [HG, NKC, NJ, P, TB] fp16 (k,q blocked)
        sums = res.results[core]["sums"].reshape(HG, T)
        for h in range(HG):
            recip = (1.0 / sums[h]).astype(np.float32)
            blk = shard[h].transpose(1, 4, 0, 3, 2).reshape(T, T)  # -> [q, k]
            np.multiply(blk, recip[:, None], out=attn[b, 8 * g + h])
    return out, attn
